# revision 31
# baseline (speedup 1.0000x reference)
"""Multi-head causal attention (b=4, l=2048, d=1024, 16 heads x 64) on 8 trn2 cores.

Sharding: core c handles batch (c // 2) and head-group (c % 2) of 8 heads.
Each core computes a partial output x[b] @ W (its 8 heads' contribution);
the host sums the two partials per batch.

v2 design (vs baseline):
  - bf16 matmul inputs everywhere (f32 PSUM accumulation).
  - causal-restricted column ranges on diagonal tiles for S, exp and PV;
    the 128x128 diagonal square is masked post-exp by a bf16 multiply on
    DVE (4x mode).  No full-tile mask multiplies.
  - z-merged [128,1024] two-bank PSUM tiles: one exp instruction covers
    both heads of a pair (160 exps instead of 320).
  - softmax denominator via a ones-column in V (O'^T row 64), normalized
    with reciprocal_approx_fast + gpsimd partition_broadcast + DVE mult.
  - Scalar engine does ONLY exp; all PSUM evacuations are on DVE.
  - software-pipelined schedule: QKV projection of l-chunk j+1 and the
    output projection of q-chunk j-1 are interleaved as filler PE work
    inside attention j's S->exp->PV pipeline so the PE never idles (and
    stays at its max pstate).
"""

import sys

sys.path.insert(0, "/opt/trn_rl_repo")

import numpy as np

import concourse.bacc as bacc
import concourse.mybir as mybir
import concourse.tile as tile
from concourse.bass_utils import run_bass_kernel_spmd

F32 = mybir.dt.float32
BF = mybir.dt.bfloat16
AF = mybir.ActivationFunctionType
ALU = mybir.AluOpType

B, L, D = 4, 2048, 1024
N_HEAD, KEY_DIM = 16, 64
HG = 8               # heads per core (head-group)
C = HG * KEY_DIM     # 512 per-core qkv width
SCALE = 1.0 / 8.0    # 1/sqrt(KEY_DIM)
NJ = 4               # q chunks of 512
ND = 8               # d chunks of 128

_CACHED = {}


def build_nc():
    nc = bacc.Bacc("TRN2", target_bir_lowering=False, debug=False)

    xT = nc.dram_tensor("xT", [D, L], BF, kind="ExternalInput")
    wq = nc.dram_tensor("wq", [D, C], BF, kind="ExternalInput")
    wk = nc.dram_tensor("wk", [D, C], BF, kind="ExternalInput")
    wv = nc.dram_tensor("wv", [D, C], BF, kind="ExternalInput")
    wo = nc.dram_tensor("wo", [C, D], BF, kind="ExternalInput")
    msk = nc.dram_tensor("msk", [128, 256], BF, kind="ExternalInput")
    out = nc.dram_tensor("out", [L, D], BF, kind="ExternalOutput")

    with tile.TileContext(nc) as tc, \
         tc.tile_pool(name="persist", bufs=1) as persist, \
         tc.tile_pool(name="const", bufs=1) as constp, \
         tc.tile_pool(name="xt", bufs=16) as xtp, \
         tc.tile_pool(name="pp", bufs=4) as pp, \
         tc.tile_pool(name="osb", bufs=2) as osbp, \
         tc.tile_pool(name="rp", bufs=4) as rp, \
         tc.tile_pool(name="ovp", bufs=4) as ovp, \
         tc.tile_pool(name="bcp", bufs=4) as bcp, \
         tc.tile_pool(name="psS", bufs=2, space="PSUM") as psS, \
         tc.tile_pool(name="psA", bufs=2, space="PSUM") as psA, \
         tc.tile_pool(name="psO", bufs=2, space="PSUM") as psO:

        qT = [persist.tile([128, L], BF, name=f"qT{t}") for t in range(4)]
        kT = [persist.tile([128, L], BF, name=f"kT{t}") for t in range(4)]
        OF = [persist.tile([128, L], BF, name=f"of{t}") for t in range(4)]
        vp = [persist.tile([128, HG, KEY_DIM + 1], BF, name=f"vp{i}")
              for i in range(16)]

        msk_sb = constp.tile([128, 256], BF, name="msk")
        wq_sb = [constp.tile([128, C], BF, name=f"wq{d}") for d in range(ND)]
        wk_sb = [constp.tile([128, C], BF, name=f"wk{d}") for d in range(ND)]
        wv_sb = [constp.tile([128, C], BF, name=f"wv{d}") for d in range(ND)]
        wo_sb = [constp.tile([128, D], BF, name=f"wo{t}") for t in range(4)]

        xts = {}

        def emit_xt_dmas(lc):
            ts = []
            for d in range(ND):
                t = xtp.tile([128, 512], BF, name=f"xt{lc}_{d}", tag="xt")
                nc.sync.dma_start(
                    t[:], xT[128 * d:128 * (d + 1), 512 * lc:512 * (lc + 1)])
                ts.append(t)
            xts[lc] = ts

        def emit_qkv_half(lc, kind, half_i):
            """One 512-col projection accumulation (8 matmuls) + evacuation,
            in its own single-bank PSUM tile."""
            ps = psA.tile([128, 512], F32, name=f"ps_{kind}{lc}{half_i}",
                          tag="psA")
            ls = slice(512 * lc, 512 * (lc + 1))
            x8 = xts[lc]
            if kind in ("q", "k"):
                cc = half_i
                w_sb = wq_sb if kind == "q" else wk_sb
                for d in range(ND):
                    nc.tensor.matmul(
                        ps[:], w_sb[d][:, 128 * cc:128 * (cc + 1)],
                        x8[d][:], start=(d == 0), stop=(d == ND - 1))
                dst = (qT if kind == "q" else kT)[cc]
                nc.vector.tensor_scalar_add(dst[:, ls], ps[:], 0.0)
            else:
                lcc = half_i
                i = 4 * lc + lcc
                for d in range(ND):
                    nc.tensor.matmul(
                        ps[:], x8[d][:, 128 * lcc:128 * (lcc + 1)],
                        wv_sb[d][:], start=(d == 0), stop=(d == ND - 1))
                nc.vector.tensor_scalar_add(
                    vp[i][:, :, 0:KEY_DIM],
                    ps[:].rearrange("p (h c) -> p h c", h=HG), 0.0)

        QKV_HALVES = [(k, h) for k in ("q", "k", "v") for h in range(4)]

        def qkv_filler_units(lc):
            units = [lambda lc=lc: emit_xt_dmas(lc)]
            for kind, h in QKV_HALVES:
                units.append(
                    lambda k=kind, h=h: emit_qkv_half(lc, k, h))
            return units

        def outproj_unit(qc, n):
            f = psA.tile([128, 512], F32, name=f"f{qc}{n}", tag="psA")
            qs = slice(128 * qc, 128 * (qc + 1))
            ncol = slice(512 * n, 512 * (n + 1))
            for t in range(4):
                nc.tensor.matmul(f[:], OF[t][:, qs], wo_sb[t][:, ncol],
                                 start=(t == 0), stop=(t == 3))
            o = osbp.tile([128, 512], BF, name=f"ob{qc}{n}", tag="osb")
            nc.vector.tensor_scalar_add(o[:], f[:], 0.0)
            nc.sync.dma_start(out[qs, ncol], o[:])

        def outproj_partial(qc, n):
            """t=0..2 accumulation only (runs before OF[3] is ready)."""
            f = psA.tile([128, 512], F32, name=f"f{qc}{n}", tag="psA")
            qs = slice(128 * qc, 128 * (qc + 1))
            ncol = slice(512 * n, 512 * (n + 1))
            for t in range(3):
                nc.tensor.matmul(f[:], OF[t][:, qs], wo_sb[t][:, ncol],
                                 start=(t == 0), stop=False)
            return f

        def outproj_final(qc, n, f):
            """t=3 accumulation + store."""
            qs = slice(128 * qc, 128 * (qc + 1))
            ncol = slice(512 * n, 512 * (n + 1))
            nc.tensor.matmul(f[:], OF[3][:, qs], wo_sb[3][:, ncol],
                             start=False, stop=True)
            o = osbp.tile([128, 512], BF, name=f"ob{qc}{n}", tag="osb")
            nc.vector.tensor_scalar_add(o[:], f[:], 0.0)
            nc.sync.dma_start(out[qs, ncol], o[:])

        def outproj_filler_units(j):
            units = []
            for qc in range(4 * j, 4 * j + 4):
                for n in (0, 1):
                    units.append(lambda q=qc, n=n: outproj_unit(q, n))
            return units

        # ---- attention emission ----
        def emit_attn(j, fillers, tail_units=()):
            n_i = 4 * j + 4
            slots = [(hp, i) for hp in range(4) for i in range(n_i)]
            ns = len(slots)
            fill_at = [[] for _ in range(ns)]
            for k, f in enumerate(fillers):
                fill_at[min(ns - 1, (k * ns) // max(1, len(fillers)))].append(f)

            p_tiles = {}
            o_ps = {}
            mask_ap = msk_sb[:].rearrange("p (z q) -> p z q", z=2)

            def emit_S_exp(hp, i):
                di = i - 4 * j
                off = 128 * di if di > 0 else 0
                s_ps = psS.tile([128, 1024], F32, name=f"s{j}{hp}{i}",
                                tag="psS")
                for z in (0, 1):
                    nc.tensor.matmul(
                        s_ps[:, 512 * z + off:512 * (z + 1)],
                        kT[hp][64 * z:64 * z + 64, 128 * i:128 * (i + 1)],
                        qT[hp][64 * z:64 * z + 64,
                               512 * j + off:512 * (j + 1)],
                        start=True, stop=True)
                p = pp.tile([128, 1024], BF, name=f"p{j}{hp}{i}", tag="pp")
                if off:
                    sap = s_ps[:].rearrange("p (z q) -> p z q", z=2)[:, :, off:]
                    pap = p[:].rearrange("p (z q) -> p z q", z=2)[:, :, off:]
                else:
                    sap, pap = s_ps[:], p[:]
                nc.scalar.activation(pap, sap, AF.Exp, scale=SCALE)
                if di >= 0:  # mask the diagonal 128x128 square (both z)
                    blk = p[:].rearrange("p (z q) -> p z q", z=2)[
                        :, :, off:off + 128]
                    nc.vector.tensor_tensor(blk, blk, mask_ap, op=ALU.mult)
                p_tiles[(hp, i)] = (p, off)

            def emit_PV(hp, i):
                p, off = p_tiles.pop((hp, i))
                if i == 0:
                    for z in (0, 1):
                        o_ps[(hp, z)] = psO.tile(
                            [65, 512], F32, name=f"o{j}{hp}{z}", tag="psO")
                for z in (0, 1):
                    nc.tensor.matmul(
                        o_ps[(hp, z)][:, off:],
                        vp[i][:, 2 * hp + z, :],
                        p[:, 512 * z + off:512 * (z + 1)],
                        start=(i == 0), stop=(i == n_i - 1))

            def emit_normalize(hp):
                js = slice(512 * j, 512 * (j + 1))
                for z in (0, 1):
                    o = o_ps.pop((hp, z))
                    # single evacuation copy frees the PSUM bank fast; the
                    # recip/broadcast/mult chain then runs from SBUF off the
                    # accumulator's critical path.  Custom-DVE ops mis-read
                    # operands at partition offset != 0, so the sums row is
                    # staged into a [1,512] tile before the recip.
                    ov = ovp.tile([65, 512], F32, name=f"ov{j}{hp}{z}",
                                  tag="ov")
                    nc.vector.tensor_scalar_add(ov[:], o[:], 0.0)
                    sr = rp.tile([1, 512], F32, name=f"sr{j}{hp}{z}", tag="sr")
                    nc.vector.tensor_scalar_add(sr[:], ov[64:65, :], 0.0)
                    r = rp.tile([1, 512], F32, name=f"r{j}{hp}{z}", tag="r")
                    nc.vector.reciprocal_approx_fast(r[:], sr[:])
                    bc = bcp.tile([64, 512], F32, name=f"bc{j}{hp}{z}",
                                  tag="bc")
                    nc.gpsimd.partition_broadcast(bc[:], r[:])
                    nc.vector.tensor_tensor(
                        OF[hp][64 * z:64 * z + 64, js], ov[0:64, :], bc[:],
                        op=ALU.mult)

            for t in range(ns + 2):
                if t < ns:
                    emit_S_exp(*slots[t])
                if t >= ns and t - ns < len(tail_units):
                    # drain-slot PE fill, placed BEFORE the stalled PVs
                    tail_units[t - ns]()
                if t >= 2:
                    hp2, i2 = slots[t - 2]
                    emit_PV(hp2, i2)
                    if i2 == n_i - 1:
                        emit_normalize(hp2)
                if t < ns:
                    for f in fill_at[t]:
                        f()

        # ---- prologue: QKV(0) with the first q-half's matmuls interleaved
        # with exactly their DMAs, so the PE starts after two transfers ----
        xts[0] = []
        ps00 = psA.tile([128, 512], F32, name="ps_q00", tag="psA")
        for d in range(ND):
            t = xtp.tile([128, 512], BF, name=f"xt0_{d}", tag="xt")
            nc.sync.dma_start(t[:], xT[128 * d:128 * (d + 1), 0:512])
            xts[0].append(t)
            nc.gpsimd.dma_start(wq_sb[d][:], wq[128 * d:128 * (d + 1), :])
            nc.tensor.matmul(ps00[:], wq_sb[d][:, 0:128], t[:],
                             start=(d == 0), stop=(d == ND - 1))
        nc.vector.tensor_scalar_add(qT[0][:, 0:512], ps00[:], 0.0)
        for d in range(ND):
            nc.gpsimd.dma_start(wk_sb[d][:], wk[128 * d:128 * (d + 1), :])
        for h in (1, 2, 3):
            emit_qkv_half(0, "q", h)
        for d in range(ND):
            nc.gpsimd.dma_start(wv_sb[d][:], wv[128 * d:128 * (d + 1), :])
        nc.gpsimd.dma_start(msk_sb[:], msk[:, :])
        for i in range(16):
            # ones column at [:, :, 64]; v evac overwrites cols 0..63
            nc.vector.memset(vp[i][:], 1.0)
        for h in range(4):
            emit_qkv_half(0, "k", h)
        for t in range(4):
            nc.gpsimd.dma_start(wo_sb[t][:], wo[128 * t:128 * (t + 1), :])
        for h in range(4):
            emit_qkv_half(0, "v", h)

        # ---- main: attention j with filler PE work chosen to balance each
        # phase against its exp load: QKV(j+1) + late-shifted outprojs ----
        OP_AT = {2: [0], 3: [1, 2]}  # attn(j) -> outproj chunks to interleave
        pend = []

        def tail_partial(qc, n):
            pend.append((qc, n, outproj_partial(qc, n)))

        for j in range(NJ):
            fillers = []
            if j + 1 < NJ:
                fillers += qkv_filler_units(j + 1)
            for jo in OP_AT.get(j, []):
                fillers += outproj_filler_units(jo)
            tails = ()
            if j == NJ - 1:
                tails = (lambda: tail_partial(12, 0),
                         lambda: tail_partial(12, 1))
            emit_attn(j, fillers, tails)

        # ---- epilogue: outproj(3); first two units were partially
        # accumulated during attn(3)'s drain slots ----
        done = set()
        for qc, n, f in pend:
            outproj_final(qc, n, f)
            done.add((qc, n))
        for qc in range(12, 16):
            for n in (0, 1):
                if (qc, n) in done:
                    continue
                outproj_final(qc, n, outproj_partial(qc, n))

    nc.finalize()
    return nc


def _get_nc():
    if "nc" not in _CACHED:
        _CACHED["nc"] = build_nc()
    return _CACHED["nc"]


def _host_mask():
    m = np.arange(128)
    q = np.arange(128)
    keep = (q[None, :] > m[:, None]).astype(np.float32)  # strict upper
    return np.concatenate([keep, keep], axis=1)  # [128, 256]


def kernel(x, W_q, W_k, W_v, W_out, trace=False, trace_kwargs=None):
    import ml_dtypes
    bf16 = ml_dtypes.bfloat16

    x = np.asarray(x, dtype=np.float32)
    W_q = np.asarray(W_q, dtype=np.float32)
    W_k = np.asarray(W_k, dtype=np.float32)
    W_v = np.asarray(W_v, dtype=np.float32)
    W_out = np.asarray(W_out, dtype=np.float32)

    nc = _get_nc()
    mask_np = _host_mask().astype(bf16)
    in_maps = []
    for core in range(8):
        b, g = core // 2, core % 2
        cs = slice(C * g, C * (g + 1))
        in_maps.append({
            "xT": np.ascontiguousarray(x[b].T).astype(bf16),
            "wq": np.ascontiguousarray(W_q[:, cs]).astype(bf16),
            "wk": np.ascontiguousarray(W_k[:, cs]).astype(bf16),
            "wv": np.ascontiguousarray(W_v[:, cs]).astype(bf16),
            "wo": np.ascontiguousarray(W_out[cs, :]).astype(bf16),
            "msk": mask_np,
        })
    res = run_bass_kernel_spmd(nc, in_maps, core_ids=list(range(8)),
                               trace=trace, **(trace_kwargs or {}))
    out = np.empty((B, L, D), dtype=np.float32)
    for b in range(B):
        out[b] = (res.results[2 * b]["out"].astype(np.float32)
                  + res.results[2 * b + 1]["out"].astype(np.float32))
        # q=0 is fully masked -> reference softmax gives uniform attention over
        # all of V; the device leaves garbage in that row, patch it here.
        out[b, 0, :] = (x[b].mean(axis=0) @ W_v) @ W_out
    if trace:
        return out, res
    return out
